# revision 1
# baseline (speedup 1.0000x reference)
"""Multi-head attention forward on 8 Trainium2 NeuronCores (Bass/Tile).

Problem: B=2, S=2048, HIDDEN=2048, HEADS=16, D_K=128, fp32 I/O,
mask all-ones (eval). torch-Linear convention: y = x @ W.T.

Sharding (head + output-row parallel, two AllToAlls, no all-reduce):
  - core c (0..7) owns heads {2c, 2c+1} for BOTH batches.
  - Phase A (per batch): project Q,K into the transposed [d, s] layout
    (row-streaming, 8 psum accumulators); project V DIRECTLY into the
    natural [s, d] layout (lhsT = vT seq-chunk, rhs = W_v slice) in two
    half-row passes — no PE transposes at all.
  - Phase B (per batch): a FLAT software pipeline over all (head,
    q-block, kt-group) groups: scoresT tiles [k, q] via Kh-stationary
    matmuls into WIDE multi-bank PSUM regions (3 k-tiles side by side),
    one wide EXP activation per region (the ACT fixed cost of ~352
    cycles amortizes 3x, keeping the scalar engine at the PE's pace),
    PV accumulation in the transposed layout, 1-group lookahead so the
    exp queue never drains across iteration boundaries. Softmax
    denominators: running DVE adds of the wide exp tiles + column folds
    + a GPSIMD partition_all_reduce (off the critical path); the LAST
    iteration instead uses a pairwise 512-wide tree + a PE ones-matmul
    partition-reduce, because its tail sits on the next phase's
    critical path (psum pool reuse is coarse-grained).  Softmax without
    max-subtraction (scores are O(few); same math as the reference).
  - One AllToAll per batch (8 ranks, 1MB, gpsimd-triggered at phase
    end — a collective trigger blocks its engine queue until
    completion, so nothing time-critical may queue behind it): A2A#0
    hides under phase A of batch 1, A2A#1 under batch 0's output
    projection; the a2a_out gather is 16 plain chunk DMAs so phase D
    starts one chunk after the collective lands.
  - Phase D: out_chunk = concat_chunk @ W_o.T per batch.
Phase order: A0 B0 [a2a0] A1 B1 [a2a1] D0 D1.
Queue discipline (head-of-line blocking is the main hazard): input
rows on sync; weights just-in-time inside the row stream (wq on scalar
at t=0, wk/wv mid-stream on sync, wo on gpsimd at B0 start); scatters
and gathers on gpsimd; output stores split scalar/sync.
Host side: pre-transpose/cast inputs to bf16 (vT half-major so the V
half-row reads are contiguous), slice weights per core, scatter-gather
the per-core [512, 2048] fp32 chunks into the full output.
"""

import math
from contextlib import ExitStack

import ml_dtypes
import numpy as np

import concourse.bass as bass
import concourse.bass_isa as bass_isa
import concourse.tile as tile
from concourse import bacc, mybir
from concourse.bass_utils import run_bass_kernel_spmd

BF16 = mybir.dt.bfloat16
F32 = mybir.dt.float32
NPBF16 = ml_dtypes.bfloat16

HIDDEN = 2048
HEADS = 16
D_K = 128
B = 2
N_CORES = 8
HPC = HEADS // N_CORES          # heads per core (2)
DPC = HPC * D_K                 # concat cols per core (256)
NHT = HIDDEN // 128             # 16 hidden-dim 128-tiles


def _mha_kernel(ctx: ExitStack, tc: tile.TileContext, aps: dict, S: int):
    nc = tc.nc
    NKT = S // 128                   # seq 128-tiles (16)
    SBLK = min(512, S)               # matmul moving-dim block
    NSB = S // SBLK                  # 4
    QBLK = SBLK
    NQB = NSB
    SCB = S // N_CORES               # per-batch output rows per core (256)
    OBLK = 512
    NOB = HIDDEN // OBLK
    NST = SCB // 128                 # 2
    scale = 1.0 / math.sqrt(D_K)
    # phase-B kt groups: (start_kt, count); alternating psum slots A/B
    GRP = [(0, 3), (3, 3), (6, 3), (9, 3), (12, 3), (15, 1)]

    qT, kT, vT = aps["qT"], aps["kT"], aps["vT"]   # per batch [HIDDEN, S]
    wqT, wkT, wvT = aps["wqT"], aps["wkT"], aps["wvT"]  # [128, NHT*DPC]
    woT = aps["woT"]                                # [128, NHT*HIDDEN]
    out = aps["out"]                                # [B*SCB, HIDDEN] f32
    a2a_in = aps["a2a_in"]                          # per batch [8*DPC, SCB]
    a2a_out = aps["a2a_out"]                        # per batch [8*DPC, SCB]

    # ---- resident weights (pre-tiled on host) ----
    w_pool = ctx.enter_context(tc.tile_pool(name="wqkv", bufs=1))
    wq_sb = w_pool.tile([128, NHT * DPC], BF16, tag="wq")
    wk_sb = w_pool.tile([128, NHT * DPC], BF16, tag="wk")
    wv_sb = w_pool.tile([128, NHT * DPC], BF16, tag="wv")
    wo_sb = w_pool.tile([128, NHT * HIDDEN], BF16, tag="wo")

    # ---- projection outputs: batch 1 reuses batch 0's slots ----
    proj_pool = ctx.enter_context(tc.tile_pool(name="proj", bufs=1))

    # ---- persistent SBUF pools (ctx scope, no cross-phase space WAR) ----
    xrow_pool = ctx.enter_context(tc.tile_pool(name="xrow", bufs=4))
    vrow_pool = ctx.enter_context(tc.tile_pool(name="vrow", bufs=4))
    es_pool = ctx.enter_context(tc.tile_pool(name="es", bufs=7))
    acc_pool = ctx.enter_context(tc.tile_pool(name="acc", bufs=2))
    fld_pool = ctx.enter_context(tc.tile_pool(name="fld", bufs=2))
    rb_pool = ctx.enter_context(tc.tile_pool(name="rb", bufs=2))
    ao_pool = ctx.enter_context(tc.tile_pool(name="ao", bufs=2))
    osb_pool = ctx.enter_context(tc.tile_pool(name="osb", bufs=2))

    qh_sb = [None] * B
    kh_sb = [None] * B
    vh_sb = [None] * B

    # all-ones stationary tile for the PE partition-reduce of the last
    # iteration's softmax denominator (short critical tail at phase ends)
    ones_pool = ctx.enter_context(tc.tile_pool(name="ones", bufs=1))
    ones_sb = ones_pool.tile([128, 128], BF16, tag="ones")
    nc.vector.memset(ones_sb[:], 1.0)

    def phase_a(b):
        qh_sb[b] = proj_pool.tile([128, HPC * S], BF16, tag="qh",
                                  name=f"qh{b}")
        kh_sb[b] = proj_pool.tile([128, HPC * S], BF16, tag="kh",
                                  name=f"kh{b}")
        vh_sb[b] = proj_pool.tile([128, NKT * DPC], BF16, tag="vh",
                                  name=f"vh{b}")
        with tc.tile_pool(name="psA", bufs=8, space="PSUM") as psA:
            # Q / K: transposed [d, s] layout, 8 psum accumulators each.
            for wi, (src, w_sb, dst) in enumerate(
                    ((qT[b], wq_sb, qh_sb[b]), (kT[b], wk_sb, kh_sb[b]))):
                ps = [psA.tile([128, SBLK], F32, tag="psA", name=f"ps{wi}_{i}")
                      for i in range(HPC * NSB)]
                for ht in range(NHT):
                    row = xrow_pool.tile([128, S], BF16, tag="xrow")
                    nc.sync.dma_start(out=row[:],
                                      in_=src[ht * 128:(ht + 1) * 128, :])
                    # weight DMAs ride the same sync queue, placed in the
                    # row stream just-in-time so they never compete with
                    # the rows the PE is about to consume.
                    if b == 0 and wi == 0 and ht == 12:
                        nc.sync.dma_start(out=wk_sb[:], in_=wkT[:, :])
                    if b == 0 and wi == 1 and ht == 4:
                        nc.sync.dma_start(out=wv_sb[:], in_=wvT[:, :])
                    for dt in range(HPC):
                        for sb in range(NSB):
                            nc.tensor.matmul(
                                ps[dt * NSB + sb][:],
                                lhsT=w_sb[:, ht * DPC + dt * 128:
                                          ht * DPC + (dt + 1) * 128],
                                rhs=row[:, sb * SBLK:(sb + 1) * SBLK],
                                start=(ht == 0), stop=(ht == NHT - 1))
                for dt in range(HPC):
                    for sb in range(NSB):
                        nc.vector.tensor_copy(
                            dst[:, dt * S + sb * SBLK: dt * S + (sb + 1) * SBLK],
                            ps[dt * NSB + sb][:])

            # V directly in natural [s, d] layout: lhsT = vT seq-chunk
            # [128h, 128s], rhs = W_v ht-block [128h, 256d].  Two
            # half-row passes (16 accumulators don't fit 8 psum banks,
            # and a matmul's start bit zeroes its whole bank, so banks
            # can't be shared).  vT is half-major on the host, so each
            # half-row read is one contiguous 256KB slab.
            SH = S // 2
            for half in range(2):
                psv = [psA.tile([128, DPC], F32, tag="psA",
                                name=f"psv{half}_{i}") for i in range(8)]
                for ht in range(NHT):
                    vrow = vrow_pool.tile([128, SH], BF16, tag="vrow")
                    nc.sync.dma_start(
                        out=vrow[:],
                        in_=vT[b][half * HIDDEN + ht * 128:
                                  half * HIDDEN + (ht + 1) * 128, :])
                    for sti in range(8):
                        nc.tensor.matmul(
                            psv[sti][:],
                            lhsT=vrow[:, sti * 128:(sti + 1) * 128],
                            rhs=wv_sb[:, ht * DPC:(ht + 1) * DPC],
                            start=(ht == 0), stop=(ht == NHT - 1))
                for sti in range(8):
                    st = half * 8 + sti
                    nc.vector.tensor_copy(
                        vh_sb[b][:, st * DPC:(st + 1) * DPC], psv[sti][:])

    def phase_b(b):
        """Flat software pipeline over all (qb, l, g) groups of a batch:
        PE stream s(k), s(k+1), PV(k), s(k+2), PV(k+1), ... crosses
        iteration boundaries, so the scalar-engine EXP queue never
        drains and the PE never waits out a pipeline refill."""
        with tc.tile_pool(name="pssA", bufs=1, space="PSUM") as psA_pool, \
             tc.tile_pool(name="pssB", bufs=1, space="PSUM") as psB_pool, \
             tc.tile_pool(name="pspv", bufs=2, space="PSUM") as pv_pool:
            NG = len(GRP)
            # l-outer: head l's AllToAll fires when its half of the
            # batch finishes — the first collective hides completely
            # under the second half's compute.
            groups = [(qb, l, g) for l in range(HPC) for qb in range(NQB)
                      for g in range(NG)]
            st = {}  # (qb, l) -> iteration state

            def state(qb, l):
                key = (qb, l)
                if key not in st:
                    st[key] = {
                        "pv": pv_pool.tile([128, QBLK], F32, tag="pv",
                                           name=f"pv{qb}_{l}"),
                        "ess": [None] * NG,
                        "acc": None,
                    }
                return st[key]

            def emit_scores(qb, l, g):
                it = state(qb, l)
                k0, n = GRP[g]
                pool = psA_pool if g % 2 == 0 else psB_pool
                w = pool.tile([128, 3 * QBLK], F32, tag="w",
                              name=f"wide{qb}_{l}_{g}")
                it[f"w{g}"] = w
                rhs_q = qh_sb[b][:, l * S + qb * QBLK: l * S + (qb + 1) * QBLK]
                for j in range(n):
                    kt = k0 + j
                    nc.tensor.matmul(
                        w[:, j * QBLK:(j + 1) * QBLK],
                        lhsT=kh_sb[b][:, l * S + kt * 128:
                                      l * S + (kt + 1) * 128],
                        rhs=rhs_q, start=True, stop=True)

            def emit_act(qb, l, g):
                it = state(qb, l)
                n = GRP[g][1]
                es = es_pool.tile([128, 3 * QBLK], BF16, tag="es",
                                  name=f"es{qb}_{l}_{g}")
                it["ess"][g] = es
                nc.scalar.activation(
                    es[:, :n * QBLK], it[f"w{g}"][:, :n * QBLK],
                    mybir.ActivationFunctionType.Exp, scale=scale)

            def emit_pv(qb, l, g):
                it = state(qb, l)
                k0, n = GRP[g]
                for j in range(n):
                    kt = k0 + j
                    nc.tensor.matmul(
                        it["pv"][:],
                        lhsT=vh_sb[b][:, kt * DPC + l * 128:
                                      kt * DPC + (l + 1) * 128],
                        rhs=it["ess"][g][:, j * QBLK:(j + 1) * QBLK],
                        start=(kt == 0), stop=(kt == NKT - 1))

            def emit_add(qb, l, g):
                it = state(qb, l)
                if qb == NQB - 1 and l == HPC - 1:
                    # last iteration: pairwise tree over 512-wide slices
                    # (alternating DVE/gpsimd) so the post-ACT critical
                    # chain is only ~log2(16) adds before the
                    # ones-matmul + recip + mul tail.
                    if g == 5:
                        return
                    pend = it.setdefault("pend", {})
                    cnt = it.setdefault("cnt", [0])

                    def fold(node):
                        level = 0
                        while level in pend:
                            cnt[0] += 1
                            nxt = acc_pool.tile([128, QBLK], BF16,
                                                tag="a5", bufs=5,
                                                name=f"a5_{cnt[0]}")
                            nc.vector.tensor_add(nxt[:], pend.pop(level)[:],
                                                 node[:])
                            node = nxt
                            level += 1
                        pend[level] = node

                    es = it["ess"][g]
                    for j in range(GRP[g][1]):
                        fold(es[:, j * QBLK:(j + 1) * QBLK])
                    if g == 4:
                        # pre-collapse pending levels off the critical
                        # path: the post-ACT5 tail is then a single add.
                        nodes = [pend.pop(k) for k in sorted(pend)]
                        node = nodes[0]
                        for other in nodes[1:]:
                            cnt[0] += 1
                            nxt = acc_pool.tile([128, QBLK], BF16,
                                                tag="a5", bufs=5,
                                                name=f"a5_{cnt[0]}")
                            nc.vector.tensor_add(nxt[:], node[:], other[:])
                            node = nxt
                        pend[0] = node
                    return
                if g == 0 or g == 5:
                    return
                a = acc_pool.tile([128, 3 * QBLK], BF16, tag="acc",
                                  name=f"acc{qb}_{l}_{g}")
                if g == 1:
                    nc.vector.tensor_add(a[:], it["ess"][0][:],
                                         it["ess"][1][:])
                else:
                    nc.vector.tensor_add(a[:], it["acc"][:], it["ess"][g][:])
                it["acc"] = a

            def emit_tail(qb, l):
                # fold 1536 -> 512 (+ last group), partition reduce,
                # reciprocal, normalize, scatter to a2a_in.
                it = state(qb, l)
                last = (qb == NQB - 1 and l == HPC - 1)
                rb = rb_pool.tile([128, QBLK], F32, tag="rb")
                if last:
                    # short tail: collapse the pairwise-tree levels plus
                    # the final slice, then a PE ones-matmul
                    # partition-reduce (into a dead wide slot) + recip —
                    # this tail sits on the next phase's critical path.
                    pend = it["pend"]
                    node = it["ess"][5][:, 0:QBLK]
                    nodes = [pend[k] for k in sorted(pend)]
                    f3b = None
                    for i, other in enumerate(nodes):
                        f3b = fld_pool.tile([128, QBLK], BF16, tag="f3b",
                                            bufs=2, name=f"f3b{i}")
                        nc.vector.tensor_add(f3b[:], node[:], other[:])
                        node = f3b
                    rbp = psB_pool.tile([128, 3 * QBLK], F32, tag="w",
                                        name="rbp")
                    nc.tensor.matmul(rbp[:, 0:QBLK], lhsT=ones_sb[:],
                                     rhs=f3b[:], start=True, stop=True)
                    nc.vector.reciprocal_approx_fast(rb[:], rbp[:, 0:QBLK])
                else:
                    a4 = it["acc"]
                    f1 = fld_pool.tile([128, QBLK], BF16, tag="f1")
                    nc.vector.tensor_add(f1[:], a4[:, 0:QBLK],
                                         a4[:, QBLK:2 * QBLK])
                    f2 = fld_pool.tile([128, QBLK], BF16, tag="f2")
                    nc.vector.tensor_add(f2[:], f1[:],
                                         a4[:, 2 * QBLK:3 * QBLK])
                    f3 = fld_pool.tile([128, QBLK], F32, tag="f3")
                    nc.vector.tensor_add(f3[:], f2[:], it["ess"][5][:, 0:QBLK])
                    nc.gpsimd.partition_all_reduce(
                        rb[:], f3[:], channels=128,
                        reduce_op=bass_isa.ReduceOp.add)
                    nc.vector.reciprocal_approx_fast(rb[:], rb[:])
                ao = ao_pool.tile([128, QBLK], BF16, tag="ao")
                nc.vector.tensor_mul(ao[:], it["pv"][:], rb[:])
                q0 = qb * QBLK
                while q0 < (qb + 1) * QBLK:
                    m = q0 // SCB
                    cend = min((qb + 1) * QBLK, (m + 1) * SCB)
                    nc.gpsimd.dma_start(
                        out=a2a_in[b][m * DPC + l * 128:
                                      m * DPC + (l + 1) * 128,
                                      q0 - m * SCB: cend - m * SCB],
                        in_=ao[:, q0 - qb * QBLK: cend - qb * QBLK])
                    q0 = cend

            emit_scores(*groups[0])
            for k, grp in enumerate(groups):
                if k + 1 < len(groups):
                    emit_scores(*groups[k + 1])
                emit_act(*grp)
                emit_pv(*grp)
                emit_add(*grp)
                if grp[2] == NG - 1:
                    emit_tail(grp[0], grp[1])
    cc_tiles = []
    cc_pool = ctx.enter_context(tc.tile_pool(name="cc", bufs=B))

    def gather_cc(b):
        # single end-of-phase AllToAll: the collective trigger blocks the
        # issuing engine queue until completion, so it must have nothing
        # time-critical queued behind it.
        coll = nc.gpsimd.collective_compute(
            "AllToAll", mybir.AluOpType.bypass,
            replica_groups=[list(range(N_CORES))],
            ins=[a2a_in[b][:, :]], outs=[a2a_out[b][:, :]])
        cc_sb = cc_pool.tile([128, NHT * SCB], BF16, tag="cc", name=f"cc{b}")
        # 16 plain chunk DMAs (not one mega-rearrange): phase D's ht loop
        # consumes chunk t first, so D starts ~one chunk after the
        # collective lands instead of after the full 1MB gather.
        for t in range(NHT):
            dma = nc.gpsimd.dma_start(
                out=cc_sb[:, t * SCB:(t + 1) * SCB],
                in_=a2a_out[b][t * 128:(t + 1) * 128, :])
            tile.add_dep_helper(dma.ins, coll.ins,
                                reason="a2a_out after collective")
        cc_tiles.append(cc_sb)

    # weights go down the scalar engine's DMA queue so the sync queue
    # carries only input rows (rows must win the pre-barrier bandwidth).
    wq_ck = NHT * DPC // 4
    for ck in range(4):
        nc.scalar.dma_start(out=wq_sb[:, ck * wq_ck:(ck + 1) * wq_ck],
                            in_=wqT[:, ck * wq_ck:(ck + 1) * wq_ck])
    phase_a(0)
    # wo rides the gpsimd queue at B0 start: it drains during phase B0
    # when the sync queue (input rows) is quiet, and gpsimd's own B0
    # work leaves plenty of queue headroom.
    wo_ck = NHT * HIDDEN // 4
    for ck in range(4):
        nc.gpsimd.dma_start(out=wo_sb[:, ck * wo_ck:(ck + 1) * wo_ck],
                            in_=woT[:, ck * wo_ck:(ck + 1) * wo_ck])
    phase_b(0)
    gather_cc(0)
    phase_a(1)
    phase_b(1)
    gather_cc(1)

    # ================= Phase D: output projection (per batch) =========
    with tc.tile_pool(name="pso", bufs=8, space="PSUM") as pso_pool:
        for b in range(B):
            cc_sb = cc_tiles[b]
            for st in range(NST):
                pso = [pso_pool.tile([128, OBLK], F32, tag="pso",
                                     name=f"pso{b}_{st}_{i}")
                       for i in range(NOB)]
                for ht in range(NHT):
                    lhs = cc_sb[:, ht * SCB + st * 128:
                                ht * SCB + (st + 1) * 128]
                    for ot in range(NOB):
                        nc.tensor.matmul(
                            pso[ot][:], lhsT=lhs,
                            rhs=wo_sb[:, ht * HIDDEN + ot * OBLK:
                                      ht * HIDDEN + (ot + 1) * OBLK],
                            start=(ht == 0), stop=(ht == NHT - 1))
                for ot in range(NOB):
                    osb = osb_pool.tile([128, OBLK], F32, tag="osb")
                    # alternate engines so the end-of-kernel drain of the
                    # last block's 4 copies+stores runs two-wide.
                    if ot % 2 == 0:
                        nc.vector.tensor_copy(osb[:], pso[ot][:])
                        eng = nc.scalar
                    else:
                        nc.scalar.copy(osb[:], pso[ot][:])
                        eng = nc.sync
                    eng.dma_start(
                        out=out[b * SCB + st * 128: b * SCB + (st + 1) * 128,
                                ot * OBLK:(ot + 1) * OBLK],
                        in_=osb[:])


def build_nc(S: int):
    nc = bacc.Bacc("TRN2", target_bir_lowering=False, debug=False,
                   enable_asserts=False, num_devices=N_CORES)
    SCB = S // N_CORES
    aps = {
        "qT": [nc.dram_tensor(f"qT{b}", [HIDDEN, S], BF16,
                              kind="ExternalInput").ap() for b in range(B)],
        "kT": [nc.dram_tensor(f"kT{b}", [HIDDEN, S], BF16,
                              kind="ExternalInput").ap() for b in range(B)],
        # half-major: [2 halves, HIDDEN, S/2] flattened — contiguous
        # 256KB half-row slabs for the V projection's streaming reads
        "vT": [nc.dram_tensor(f"vT{b}", [2 * HIDDEN, S // 2], BF16,
                              kind="ExternalInput").ap() for b in range(B)],
        "wqT": nc.dram_tensor("wqT", [128, NHT * DPC], BF16,
                              kind="ExternalInput").ap(),
        "wkT": nc.dram_tensor("wkT", [128, NHT * DPC], BF16,
                              kind="ExternalInput").ap(),
        "wvT": nc.dram_tensor("wvT", [128, NHT * DPC], BF16,
                              kind="ExternalInput").ap(),
        "woT": nc.dram_tensor("woT", [128, NHT * HIDDEN], BF16,
                              kind="ExternalInput").ap(),
        "out": nc.dram_tensor("out", [B * SCB, HIDDEN], F32,
                              kind="ExternalOutput").ap(),
        "a2a_in": [nc.dram_tensor(f"a2a_in{b}", [N_CORES * DPC, SCB],
                                  BF16).ap() for b in range(B)],
        "a2a_out": [nc.dram_tensor(f"a2a_out{b}", [N_CORES * DPC, SCB],
                                   BF16).ap() for b in range(B)],
    }
    with tile.TileContext(nc) as tc:
        with ExitStack() as ctx:
            _mha_kernel(ctx, tc, aps, S)
    nc.compile()
    return nc


_NC_CACHE: dict = {}


def _tile_weight(w_slice_T):
    """[H, D] -> [128, (H//128)*D] with 128-row tiles laid out consecutively."""
    H, D = w_slice_T.shape
    return np.ascontiguousarray(
        w_slice_T.reshape(H // 128, 128, D).transpose(1, 0, 2).reshape(
            128, (H // 128) * D))


def make_in_maps(q, k, v, w_q, w_k, w_v, w_o):
    """Host-side shard/cast. Returns per-core input dicts."""
    qT = [np.ascontiguousarray(q[b].T).astype(NPBF16) for b in range(B)]
    kT = [np.ascontiguousarray(k[b].T).astype(NPBF16) for b in range(B)]
    vT = [np.ascontiguousarray(
        v[b].T.reshape(HIDDEN, 2, -1).transpose(1, 0, 2).reshape(
            2 * HIDDEN, -1)).astype(NPBF16) for b in range(B)]
    woT = _tile_weight(np.ascontiguousarray(w_o.T).astype(NPBF16))
    in_maps = []
    for c in range(N_CORES):
        d0 = c * DPC
        m = {}
        for b in range(B):
            m[f"qT{b}"] = qT[b]
            m[f"kT{b}"] = kT[b]
            m[f"vT{b}"] = vT[b]
        m["wqT"] = _tile_weight(
            np.ascontiguousarray(w_q[d0:d0 + DPC, :].T).astype(NPBF16))
        m["wkT"] = _tile_weight(
            np.ascontiguousarray(w_k[d0:d0 + DPC, :].T).astype(NPBF16))
        m["wvT"] = _tile_weight(
            np.ascontiguousarray(w_v[d0:d0 + DPC, :].T).astype(NPBF16))
        m["woT"] = woT
        in_maps.append(m)
    return in_maps


def kernel(q, k, v, mask, w_q, w_k, w_v, w_o, _trace=False):
    q = np.asarray(q, np.float32)
    k = np.asarray(k, np.float32)
    v = np.asarray(v, np.float32)
    mask = np.asarray(mask)
    w_q = np.asarray(w_q, np.float32)
    w_k = np.asarray(w_k, np.float32)
    w_v = np.asarray(w_v, np.float32)
    w_o = np.asarray(w_o, np.float32)
    S = q.shape[1]

    if not np.all(mask != 0):
        # General-mask fallback (never hit for the eval problem: mask is
        # all ones).  Computed on host for correctness.
        return _numpy_reference(q, k, v, mask, w_q, w_k, w_v, w_o)

    if S not in _NC_CACHE:
        _NC_CACHE[S] = build_nc(S)
    nc = _NC_CACHE[S]

    in_maps = make_in_maps(q, k, v, w_q, w_k, w_v, w_o)
    res = run_bass_kernel_spmd(nc, in_maps, core_ids=list(range(N_CORES)),
                               trace=_trace)

    SCB = S // N_CORES
    out = np.empty((B, S, HIDDEN), np.float32)
    for c in range(N_CORES):
        for b in range(B):
            out[b, c * SCB:(c + 1) * SCB, :] = \
                res.results[c]["out"][b * SCB:(b + 1) * SCB, :]
    if _trace:
        return out, res
    return out


def _numpy_reference(q, k, v, mask, w_q, w_k, w_v, w_o):
    Bn, S, H = q.shape
    dk = H // HEADS

    def split_heads(x, w):
        y = x @ w.T
        return y.reshape(Bn, S, HEADS, dk).transpose(0, 2, 1, 3)

    qh = split_heads(q, w_q)
    kh = split_heads(k, w_k)
    vh = split_heads(v, w_v)
    s = np.einsum("bhqd,bhkd->bhqk", qh, kh) / np.sqrt(np.float32(dk))
    s = np.where(mask[:, None, :, :] == 0, np.float32(-1e9), s)
    s = s - s.max(-1, keepdims=True)
    e = np.exp(s)
    a = e / e.sum(-1, keepdims=True)
    o = np.einsum("bhqk,bhkd->bhqd", a, vh)
    o = o.transpose(0, 2, 1, 3).reshape(Bn, S, H)
    return (o @ w_o.T).astype(np.float32)

